# revision 1
# baseline (speedup 1.0000x reference)
"""Trainium2 Bass kernel for a dense transformer block (B=2, T=2048, C=1024, H=16).

Sharding over 8 NeuronCores:
  - LN / QKV / proj / MLP are row-sharded: core c owns 512 contiguous token rows
    (batch c//4, rows [512*(c%4), 512*(c%4+1))).
  - Attention is head-sharded: core c owns heads {2c%16, 2c%16+1} for BOTH
    batches (4 (batch, head) pairs per core), so the causal work is identical on
    every core and the SPMD program is rank-uniform.
  - An 8-way AllToAll distributes Q^T/K^T (bf16) from row-owners to
    head-owners, issued early so attention score matmuls overlap a second
    8-way AllToAll carrying V (f32r); a third AllToAll returns attention
    outputs y^T (f32r) to row-owners.

Matmuls run in float32r (full-speed fp32 mode, ~tf32-ish rounding) except the
attention scores, whose Q/K operands travel and multiply in bf16 (softmax
probabilities here are near-uniform, so score rounding is benign: measured
end-to-end relative error stays at 1.05e-4). The MLP intermediate (8 MB)
streams through DRAM to keep SBUF free for deep double-buffering.
"""

from contextlib import ExitStack

import numpy as np

import concourse.bacc as bacc
import concourse.bass as bass
import concourse.mybir as mybir
import concourse.tile as tile
from concourse.bass_utils import run_bass_kernel_spmd

P = 128
B, T, C, H, Dh = 2, 2048, 1024, 16, 64
NCORES = 8
R = 512          # token rows per core
RT = R // P      # 4 row tiles
CK = C // P      # 8 C-chunks
F32 = mybir.dt.float32
F32R = mybir.dt.float32r
EPS = 1e-5
SCALE = float(C) ** -0.5  # 2**-5

# AllToAll shard layouts (per destination core d):
#   qk buffer: [ qT M-tile d (128x512) | kT M-tile d (128x512) ]  (bf16)
#   v buffer:  V cols [128d,128d+128) as [p 128][chunk 4][head 2][64+ones] (f32r)
QKP = P * R                  # 65536 elems for q part (and k part)
VP = R * 2 * (Dh + 1)        # 512*130 = 66560
SH = 2 * QKP + VP            # 197632
VOFF = 2 * QKP

_CACHE = {}


def _ln_tile(nc, sm, dst, src, w, eps_t):
    """dst = layer_norm(src) * w, rows on partitions, norm over 1024 free dim."""
    stats = sm.tile([P, 2, 6], F32, tag="stats", bufs=2)
    for g in range(2):
        nc.vector.bn_stats(out=stats[:, g, :], in_=src[:, g * 512:(g + 1) * 512])
    mv = sm.tile([P, 2], F32, tag="mv", bufs=2)
    nc.vector.bn_aggr(out=mv[:], in_=stats[:])
    rstd = sm.tile([P, 1], F32, tag="rstd", bufs=2)
    nc.scalar.activation(
        out=rstd[:], in_=mv[:, 1:2], func=mybir.ActivationFunctionType.Sqrt,
        bias=eps_t[:], scale=1.0,
    )
    nc.vector.reciprocal(out=rstd[:], in_=rstd[:])
    nc.vector.tensor_scalar(
        out=dst, in0=src, scalar1=mv[:, 0:1], scalar2=rstd[:],
        op0=mybir.AluOpType.subtract, op1=mybir.AluOpType.mult,
    )
    nc.vector.tensor_mul(dst, dst, w[:])


def _transpose_to(nc, ps, ev_pool, dst, src_tiles, idt):
    """dst[P, CK, R] (f32r) = transpose of h[P, RT, C] (f32).

    src_tiles: the h tile; for each C-chunk k, 4 PE transposes fill a psum
    [128, 512] bank which is then evicted to dst[:, k, :].
    """
    for k in range(CK):
        pt = ps.tile([P, 512], F32, tag="ps")
        for r in range(RT):
            nc.tensor.matmul(
                pt[:, r * P:(r + 1) * P],
                src_tiles[:, r, k * P:(k + 1) * P],
                idt[:],
                is_transpose=True,
                start=(r == 0),
                stop=(r == RT - 1),
            )
        nc.vector.tensor_copy(dst[:, k, :], pt[:])


def build():
    nc = bacc.Bacc(None, target_bir_lowering=False)

    # host-pretiled inputs (see _make_in_maps for layouts)
    xin = nc.declare_dram_parameter("xin", [P, RT * C], F32, isOutput=False)
    ln1w = nc.declare_dram_parameter("ln1w", [P, C], F32, isOutput=False)
    ln2w = nc.declare_dram_parameter("ln2w", [P, C], F32, isOutput=False)
    wat = nc.declare_dram_parameter("wat", [8, P, 2 * C], F32R, isOutput=False)
    wvt = nc.declare_dram_parameter("wvt", [2, 2, P, 4 * 512], F32R, isOutput=False)
    wpt = nc.declare_dram_parameter("wpt", [P, CK * C], F32R, isOutput=False)
    wft = nc.declare_dram_parameter("wft", [16, P, 2 * C], F32R, isOutput=False)
    wct = nc.declare_dram_parameter("wct", [8, P, 4 * 2 * 512], F32R, isOutput=False)
    identr = nc.declare_dram_parameter("identr", [P, P], F32R, isOutput=False)
    identf = nc.declare_dram_parameter("identf", [P, P], F32, isOutput=False)
    maskd = nc.declare_dram_parameter("maskd", [4, P, 512], mybir.dt.bfloat16, isOutput=False)
    out = nc.declare_dram_parameter("out", [R, C], F32, isOutput=True)

    with tile.TileContext(nc) as tc, ExitStack() as ctx:
        const = ctx.enter_context(tc.tile_pool(name="const", bufs=1))
        big = ctx.enter_context(tc.tile_pool(name="big", bufs=1))
        wcol = ctx.enter_context(tc.tile_pool(name="wcol", bufs=2))
        wrow = ctx.enter_context(tc.tile_pool(name="wrow", bufs=2))
        kv = ctx.enter_context(tc.tile_pool(name="kv", bufs=2))
        sm = ctx.enter_context(tc.tile_pool(name="sm", bufs=4))
        ev = ctx.enter_context(tc.tile_pool(name="ev", bufs=2))
        ps = ctx.enter_context(tc.tile_pool(name="ps", bufs=8, space="PSUM"))
        dram = ctx.enter_context(tc.tile_pool(name="dram", bufs=1, space="DRAM"))

        # ---------- constants ----------
        idt = const.tile([P, P], F32R)
        nc.sync.dma_start(idt[:], identr[:])
        idtf = const.tile([P, P], F32)
        nc.sync.dma_start(idtf[:], identf[:])
        mD = const.tile([P, 4, 512], mybir.dt.bfloat16)
        nc.sync.dma_start(mD[:], maskd[:].rearrange("i p c -> p i c"))
        w1 = const.tile([P, C], F32, tag="lnw")
        nc.sync.dma_start(w1[:], ln1w[:])
        eps_t = const.tile([P, 1], F32)
        nc.any.memset(eps_t[:], EPS)
        ones_f = const.tile([P, 8], F32)
        nc.any.memset(ones_f[:], 1.0)
        ones_c = const.tile([P, 8], F32R)
        nc.vector.tensor_copy(ones_c[:], ones_f[:])

        # ---------- collective DRAM buffers ----------
        a1q_in = dram.tile([NCORES, 2 * QKP], mybir.dt.bfloat16, name="a1q_in")
        a1q_out = dram.tile([NCORES, 2 * QKP], mybir.dt.bfloat16, name="a1q_out")
        a1v_in = dram.tile([NCORES, VP], F32R, name="a1v_in")
        a1v_out = dram.tile([NCORES, VP], F32R, name="a1v_out")
        a2_in = dram.tile([NCORES, P, R], F32R, name="a2_in")
        mt_d = dram.tile([32, P, 512], F32R, name="mt_d")
        a2_out = dram.tile([NCORES, P, R], F32R, name="a2_out")

        # ---------- phase 1: load x, LN1, transpose ----------
        with nc.named_scope("ln1"):
            xres = big.tile([P, RT, C], F32, tag="x", name="xres")
            nc.sync.dma_start(xres[:].rearrange("p m c -> p (m c)"), xin[:])
            h = big.tile([P, RT, C], F32, tag="h", name="h")
            for m in range(RT):
                _ln_tile(nc, sm, h[:, m, :], xres[:, m, :], w1, eps_t)
            hT = big.tile([P, CK, R], F32R, tag="ht", name="hT")
            _transpose_to(nc, ps, ev, hT, h, idtf)

        # ---------- phase 2: qk^T matmuls -> q/k shards; V natural -> v shards ----------
        with nc.named_scope("qkv"):
            et = None
            for m in range(16):
                if m % 2 == 0:
                    wb = wcol.tile([P, 2, CK, P], F32R, tag="wc", name="wb")
                    nc.sync.dma_start(
                        wb[:].rearrange("p i k q -> p (i k q)"), wat[m // 2]
                    )
                pm = ps.tile([P, 512], F32, tag="ps")
                for k in range(CK):
                    nc.tensor.matmul(
                        pm[:], wb[:, m % 2, k, :], hT[:, k, :],
                        start=(k == 0), stop=(k == CK - 1),
                    )
                if m % 2 == 0:
                    et = ev.tile([P, C], mybir.dt.bfloat16, tag="ev", bufs=2, name="et")
                nc.vector.tensor_copy(et[:, 512 * (m % 2):512 * (m % 2) + 512], pm[:])
                if m % 2 == 1:
                    d = m % 8 - 1
                    off = 0 if m < 8 else QKP
                    dst = a1q_in[d:d + 2, off:off + QKP].rearrange(
                        "d (p c) -> p d c", c=R
                    )
                    nc.sync.dma_start(dst, et[:].rearrange("p (d c) -> p d c", c=R))

            # q/k shards complete: start their AllToAll while V is computed
            nc.gpsimd.collective_compute(
                "AllToAll",
                mybir.AluOpType.bypass,
                ins=[a1q_in[:].opt()],
                outs=[a1q_out[:].opt()],
                replica_groups=[list(range(NCORES))],
            )

            # V in natural layout [rows, vcols], half the vcols per pass
            vall = a1v_in[:].rearrange(
                "d (p cs hh x) -> p d cs hh x", p=P, cs=RT, hh=2
            )
            for half in range(2):
                pvs = []
                for kg in range(2):
                    wv = wrow.tile([P, 4, 512], F32R, tag="wr", name="wv")
                    nc.sync.dma_start(
                        wv[:].rearrange("p i c -> p (i c)"), wvt[half, kg]
                    )
                    for m in range(RT):
                        if kg == 0:
                            pvs.append(ps.tile(
                                [P, 512], F32, tag="ps", name=f"pv{half}_{m}"
                            ))
                        pvm = pvs[m]
                        for k4 in range(4):
                            k = 4 * kg + k4
                            nc.tensor.matmul(
                                pvm[:], hT[:, k, m * P:(m + 1) * P], wv[:, k4, :],
                                start=(k == 0), stop=(k == CK - 1),
                            )
                for m in range(RT):
                    vev = ev.tile([P, 512], F32R, tag="ev", bufs=2, name="vev")
                    nc.vector.tensor_copy(vev[:], pvs[m][:])
                    vv3 = vev[:].rearrange("p (dd hh x) -> p dd hh x", dd=4, x=Dh)
                    for hh in range(2):
                        nc.sync.dma_start(
                            vall[:, 4 * half:4 * half + 4, m, hh, 0:Dh],
                            vv3[:, :, hh, :],
                        )
            # ones columns: per shard, [p, cs, {64,129}] strided dest
            for d in range(NCORES):
                vsh = a1v_in[d].rearrange("(p cs y) -> p cs y", p=P, y=2 * (Dh + 1))
                nc.sync.dma_start(
                    vsh[:, :, Dh::Dh + 1],
                    ones_c[:].rearrange("p (cs hh) -> p cs hh", cs=RT),
                )

        # ---------- phase 3: AllToAll for V ----------
        nc.gpsimd.collective_compute(
            "AllToAll",
            mybir.AluOpType.bypass,
            ins=[a1v_in[:].opt()],
            outs=[a1v_out[:].opt()],
            replica_groups=[list(range(NCORES))],
        )

        # ---------- phase 4: attention (4 (batch, head) pairs per core) ----------
        with nc.named_scope("attn"):
            for p_i in range(4):
                b = p_i // 2
                hl = p_i % 2
                sb = 4 * b
                kt = kv.tile([Dh, 4, R], mybir.dt.bfloat16, tag="kt")
                nc.sync.dma_start(
                    kt[:],
                    a1q_out[sb:sb + 4, QKP + hl * Dh * R: QKP + (hl + 1) * Dh * R]
                    .rearrange("s (r c) -> r s c", c=R),
                )
                qt = kv.tile([Dh, 4, R], mybir.dt.bfloat16, tag="qt")
                nc.sync.dma_start(
                    qt[:],
                    a1q_out[sb:sb + 4, hl * Dh * R:(hl + 1) * Dh * R]
                    .rearrange("s (r c) -> r s c", c=R),
                )
                # vv: per shard s the v-region is [p 128][cs 4][hh 2][65];
                # load all 4 source shards -> [128, 4, 520]
                vv = kv.tile([P, 4, 520], F32R, tag="vv")
                nc.sync.dma_start(
                    vv[:],
                    a1v_out[sb:sb + 4, :].rearrange("s (p x) -> p s x", p=P),
                )

                for jq in range(4):
                    glast = 4 * jq + 3
                    py = ps.tile([P, 512], F32, tag="ps")
                    for g in range(glast + 1):
                        pS = ps.tile([P, 512], F32, tag="ps")
                        nc.tensor.matmul(
                            pS[:],
                            kt[:, g // 4, (g % 4) * P:(g % 4 + 1) * P],
                            qt[:, jq, :],
                            start=True, stop=True,
                        )
                        es = sm.tile([P, 512], F32R, tag="es", bufs=4)
                        if g < 4 * jq:
                            nc.scalar.activation(
                                out=es[:], in_=pS[:],
                                func=mybir.ActivationFunctionType.Exp, scale=SCALE,
                            )
                        else:
                            tmp = sm.tile([P, 512], F32, tag="etmp", bufs=2)
                            nc.scalar.activation(
                                out=tmp[:], in_=pS[:],
                                func=mybir.ActivationFunctionType.Exp, scale=SCALE,
                            )
                            nc.vector.tensor_mul(es[:], tmp[:], mD[:, g - 4 * jq, :])
                        vslice = vv[:, g // 4,
                                    (g % 4) * 130 + hl * 65:
                                    (g % 4) * 130 + hl * 65 + 65]
                        nc.tensor.matmul(
                            py[0:Dh + 1, :], vslice, es[:],
                            start=(g == 0), stop=(g == glast),
                        )
                    # normalize: y / denom (denom is psum row Dh)
                    ysf = sm.tile([Dh + 1, 512], F32, tag="ys", bufs=2)
                    nc.vector.tensor_copy(ysf[:], py[0:Dh + 1, :])
                    rr0 = sm.tile([1, 512], F32, tag="rr", bufs=2)
                    nc.sync.dma_start(rr0[:], ysf[Dh:Dh + 1, :])
                    nc.vector.reciprocal(out=rr0[:], in_=rr0[:])
                    bb = sm.tile([Dh, 512], F32, tag="bb", bufs=2)
                    nc.gpsimd.partition_broadcast(bb[:], rr0[:], channels=Dh)
                    yst = sm.tile([Dh, 512], F32R, tag="yst", bufs=2)
                    nc.vector.tensor_mul(yst[:], ysf[0:Dh, :], bb[:])
                    # strip jq covers exactly shard (4*b + jq)'s q columns
                    d = 4 * b + jq
                    nc.sync.dma_start(
                        a2_in[d, Dh * hl:Dh * (hl + 1), :], yst[:]
                    )

        # ---------- phase 5: AllToAll #2 (yT back to row owners) ----------
        nc.gpsimd.collective_compute(
            "AllToAll",
            mybir.AluOpType.bypass,
            ins=[a2_in[:].opt()],
            outs=[a2_out[:].opt()],
            replica_groups=[list(range(NCORES))],
        )

        # ---------- phase 6: proj + residual (in place into xres) ----------
        with nc.named_scope("proj"):
            yTm = big.tile([P, CK, R], F32R, tag="ht", name="yTm")
            nc.sync.dma_start(
                yTm[:], a2_out[:].rearrange("s p c -> p s c")
            )
            pps = [
                ps.tile([P, 512], F32, tag="ps", name=f"pp{i}") for i in range(8)
            ]
            for k in range(CK):
                wpk = wrow.tile([P, 2, 512], F32R, tag="wr", name="wpk")
                nc.sync.dma_start(
                    wpk[:].rearrange("p i c -> p (i c)"),
                    wpt[:, k * C:(k + 1) * C],
                )
                for m in range(RT):
                    nc.tensor.matmul(
                        pps[2 * m][:], yTm[:, k, m * P:(m + 1) * P], wpk[:, 0, :],
                        start=(k == 0), stop=(k == CK - 1),
                    )
                    nc.tensor.matmul(
                        pps[2 * m + 1][:], yTm[:, k, m * P:(m + 1) * P], wpk[:, 1, :],
                        start=(k == 0), stop=(k == CK - 1),
                    )
            for m in range(RT):
                nc.vector.tensor_add(xres[:, m, 0:512], pps[2 * m][:],
                                     xres[:, m, 0:512])
                nc.vector.tensor_add(xres[:, m, 512:1024], pps[2 * m + 1][:],
                                     xres[:, m, 512:1024])

        # ---------- phase 7: LN2 + transpose ----------
        with nc.named_scope("ln2"):
            w2 = const.tile([P, C], F32, tag="lnw", name="w2")
            nc.sync.dma_start(w2[:], ln2w[:])
            h2 = big.tile([P, RT, C], F32, tag="h", name="h2")
            for m in range(RT):
                _ln_tile(nc, sm, h2[:, m, :], xres[:, m, :], w2, eps_t)
            h2T = big.tile([P, CK, R], F32R, tag="ht", name="h2T")
            _transpose_to(nc, ps, ev, h2T, h2, idtf)

        # ---------- phase 8: fc (relu) -> mT ----------
        with nc.named_scope("mlp"):
            for m in range(32):
                if m % 2 == 0:
                    wb = wcol.tile([P, 2, CK, P], F32R, tag="wc", name="wbf")
                    nc.sync.dma_start(
                        wb[:].rearrange("p i k q -> p (i k q)"), wft[m // 2]
                    )
                pm = ps.tile([P, 512], F32, tag="ps")
                for k in range(CK):
                    nc.tensor.matmul(
                        pm[:], wb[:, m % 2, k, :], h2T[:, k, :],
                        start=(k == 0), stop=(k == CK - 1),
                    )
                mtb = sm.tile([P, 512], F32R, tag="mtb", bufs=3)
                nc.scalar.activation(
                    out=mtb[:], in_=pm[:],
                    func=mybir.ActivationFunctionType.Relu,
                )
                nc.sync.dma_start(mt_d[m], mtb[:])

            # ---------- phase 9: cproj + residual -> out ----------
            out_r = out[:].rearrange("(m p) c -> p m c", p=P)
            pcs = [
                ps.tile([P, 512], F32, tag="ps", name=f"pc{i}") for i in range(8)
            ]
            for k in range(32):
                if k % 4 == 0:
                    wk = wrow.tile([P, 4, 2, 512], F32R, tag="wr", name="wk")
                    nc.sync.dma_start(
                        wk[:].rearrange("p i h c -> p (i h c)"), wct[k // 4]
                    )
                if k % 2 == 0:
                    mtr = sm.tile([P, 2, 512], F32R, tag="mtr", bufs=3)
                    nc.sync.dma_start(
                        mtr[:], mt_d[k:k + 2].rearrange("i p c -> p i c")
                    )
                for m in range(RT):
                    for half in range(2):
                        nc.tensor.matmul(
                            pcs[2 * m + half][:],
                            mtr[:, k % 2, m * P:(m + 1) * P],
                            wk[:, k % 4, half, :],
                            start=(k == 0), stop=(k == 31),
                        )
            for m in range(RT):
                for half in range(2):
                    ot = ev.tile([P, C], F32, tag="ev", bufs=2, name="ot")
                    nc.vector.tensor_add(
                        ot[:, 0:512], pcs[2 * m + half][:],
                        xres[:, m, 512 * half:512 * half + 512]
                    )
                    nc.sync.dma_start(
                        out_r[:, m, 512 * half:512 * half + 512], ot[:, 0:512]
                    )

    nc.finalize()
    return nc


def _get_nc():
    if "nc" not in _CACHE:
        _CACHE["nc"] = build()
    return _CACHE["nc"]


def _make_in_maps(x, ln1_w, w_attn, w_proj, ln2_w, w_fc, w_cproj):
    x = np.asarray(x, dtype=np.float32)
    ln1_w = np.asarray(ln1_w, dtype=np.float32)
    ln2_w = np.asarray(ln2_w, dtype=np.float32)
    w_attn = np.asarray(w_attn, dtype=np.float32)
    w_proj = np.asarray(w_proj, dtype=np.float32)
    w_fc = np.asarray(w_fc, dtype=np.float32)
    w_cproj = np.asarray(w_cproj, dtype=np.float32)

    ln1b = np.ascontiguousarray(np.tile(ln1_w[None, :], (P, 1)))
    ln2b = np.ascontiguousarray(np.tile(ln2_w[None, :], (P, 1)))
    ident = np.eye(P, dtype=np.float32)
    ii = np.arange(P)[:, None]
    jj = np.arange(512)[None, :]
    import ml_dtypes
    maskd = np.stack(
        [(ii <= jj - P * i).astype(ml_dtypes.bfloat16) for i in range(4)]
    )  # [4, 128, 512] bf16

    # pretile weights: wat[m, p, (k q)] = w_attn[128k + p, 128m + q]
    wqk = w_attn[:, 0:2 * C]
    wat = np.ascontiguousarray(
        wqk.reshape(CK, P, 16, P).transpose(2, 1, 0, 3).reshape(8, 2, P, CK * P)
        .transpose(0, 2, 1, 3).reshape(8, P, 2 * C)
    )
    # wvt[half, kg, p, (k4 c)] = w_attn[128*(4kg+k4)+p, 2048 + 512*half + c]
    wv_ = w_attn[:, 2 * C:3 * C]
    wvt = np.ascontiguousarray(
        wv_.reshape(2, 4, P, 2, 512).transpose(3, 0, 2, 1, 4).reshape(2, 2, P, 4 * 512)
    )
    wft = np.ascontiguousarray(
        w_fc.reshape(CK, P, 32, P).transpose(2, 1, 0, 3).reshape(16, 2, P, CK * P)
        .transpose(0, 2, 1, 3).reshape(16, P, 2 * C)
    )
    # wpt[p, (k c)] = w_proj[128k + p, c]
    wpt = np.ascontiguousarray(
        w_proj.reshape(CK, P, C).transpose(1, 0, 2).reshape(P, CK * C)
    )
    # wct[half, k, p, c] = w_cproj[128k + p, 512 half + c]
    # wct[kg, p, (k4 half c)] = w_cproj[128*(4kg+k4)+p, 512*half+c]
    wct = np.ascontiguousarray(
        w_cproj.reshape(8, 4, P, 2, 512).transpose(0, 2, 1, 3, 4).reshape(8, P, 4 * 2 * 512)
    )

    in_maps = []
    for c in range(NCORES):
        b = c // 4
        r0 = 512 * (c % 4)
        xr = x[b, r0:r0 + R]  # [512, 1024]
        xt = np.ascontiguousarray(
            xr.reshape(RT, P, C).transpose(1, 0, 2).reshape(P, RT * C)
        )
        in_maps.append({
            "xin": xt,
            "ln1w": ln1b, "ln2w": ln2b,
            "wat": wat, "wvt": wvt, "wpt": wpt, "wft": wft, "wct": wct,
            "identr": ident, "identf": ident, "maskd": maskd,
        })
    return in_maps


def run(x, ln1_w, w_attn, w_proj, ln2_w, w_fc, w_cproj, trace=False):
    nc = _get_nc()
    in_maps = _make_in_maps(x, ln1_w, w_attn, w_proj, ln2_w, w_fc, w_cproj)
    res = run_bass_kernel_spmd(nc, in_maps, list(range(NCORES)), trace=trace)
    out = np.empty((B, T, C), dtype=np.float32)
    for c in range(NCORES):
        b = c // 4
        r0 = 512 * (c % 4)
        out[b, r0:r0 + R] = res.results[c]["out"]
    return out, res


def kernel(x, ln1_w, w_attn, w_proj, ln2_w, w_fc, w_cproj):
    out, _ = run(x, ln1_w, w_attn, w_proj, ln2_w, w_fc, w_cproj)
    return out



# revision 17
# speedup vs baseline: 1.5390x; 1.5390x over previous
"""Trainium2 Bass kernel for a dense transformer block (B=2, T=2048, C=1024, H=16).

Sharding over 8 NeuronCores (same topology as the f32r baseline):
  - LN / QKV / proj / MLP row-sharded: core c owns 512 contiguous token rows
    (batch c//4, rows [512*(c%4), 512*(c%4+1))).
  - Attention head-sharded: core d owns heads {2d, 2d+1} for BOTH batches
    (4 (batch, head) pairs per core); three AllToAlls re-shard q/k, v, and y.

Precision strategy (rel-err budget 2e-2; attention contributes ~2% of the
output magnitude, MLP ~35%):
  - Attention path entirely fp8: q/k/v/es/h in e4m3, y/w_proj in e5m2.
    All attention-side matmuls use the fp8 DoubleRow perf mode (2 k-tiles
    per instruction at 0.5 cycles/row).
  - MLP (fc/cproj) in bf16 -- fp8 there would cost ~2e-2 alone.
  - Causal mask is accumulated into the score psum by an fp8 matmul
    (identity-stationary, mask moving; -448*2^-5 = -14 -> exp ~ 8e-7),
    so every strip takes exactly one Exp and no mask multiply.
  - ln weights are folded into w_attn / w_fc on the host (exact identity).
  - The MLP intermediate lives in SBUF (bf16, 32 KB/partition) -- no DRAM
    roundtrip.

Collectives (fp8/fp8e5) shrink to 1 MB + 0.5 MB + 0.5 MB and overlap:
a2a#1(qk) runs over the v-matmul tail, a2a#2(v) under the score/exp stream,
and MLP weights prefetch under attention.
"""

from contextlib import ExitStack

import numpy as np

import concourse.bacc as bacc
import concourse.bass as bass
import concourse.mybir as mybir
import concourse.tile as tile
from concourse.bass_utils import run_bass_kernel_spmd

P = 128
B, T, C, H, Dh = 2, 2048, 1024, 16, 64
NCORES = 8
R = 512          # token rows per core
RT = R // P      # 4 row tiles
CK = C // P      # 8 C-chunks
F32 = mybir.dt.float32
BF16 = mybir.dt.bfloat16
F8 = mybir.dt.float8e4
F85 = mybir.dt.float8e5
DR = mybir.MatmulPerfMode.DoubleRow
EPS = 1e-5
SCALE = float(C) ** -0.5  # 2**-5
MASKV = -240.0            # e4m3 (IEEE) max finite; exp(-240/32)=5.5e-4 -> fp8 rounds to exactly 0

# a1q per-core buffer: [8 dests][qk 2][128 part][512 rows] fp8
QKP = P * R               # 65536 B per (dest, q/k)
# a1v per-core buffer: [8 dests][128 part][hh 2][cspair 2][cs2 2][64] fp8
VP = P * 512
# a2 per-core buffer: [8 dests][128 dims][512 rows] fp8e5

_CACHE = {}


def _ln_tile(nc, sm, dst, src, eps_t):
    """dst = layer_norm(src) (gain folded into the next weights)."""
    stats = sm.tile([P, 2, 6], F32, tag="stats", bufs=2)
    for g in range(2):
        nc.vector.bn_stats(out=stats[:, g, :], in_=src[:, g * 512:(g + 1) * 512])
    mv = sm.tile([P, 2], F32, tag="mv", bufs=2)
    nc.vector.bn_aggr(out=mv[:], in_=stats[:])
    rstd = sm.tile([P, 1], F32, tag="rstd", bufs=2)
    nc.scalar.activation(
        out=rstd[:], in_=mv[:, 1:2], func=mybir.ActivationFunctionType.Sqrt,
        bias=eps_t[:], scale=1.0,
    )
    nc.vector.reciprocal(out=rstd[:], in_=rstd[:])
    nc.vector.tensor_scalar(
        out=dst, in0=src, scalar1=mv[:, 0:1], scalar2=rstd[:],
        op0=mybir.AluOpType.subtract, op1=mybir.AluOpType.mult,
    )


def build():
    nc = bacc.Bacc(None, target_bir_lowering=False)

    # host-pretiled inputs (see _make_in_maps for layouts)
    xin = nc.declare_dram_parameter("xin", [P, RT * C], F32, isOutput=False)
    wqk8 = nc.declare_dram_parameter("wqk8", [16, 4, P, 2 * P], F8, isOutput=False)
    wv8 = nc.declare_dram_parameter("wv8", [2, P, 8, 512], F8, isOutput=False)
    wp8 = nc.declare_dram_parameter("wp8", [8, P, 2 * 512], F85, isOutput=False)
    wft = nc.declare_dram_parameter("wft", [16, P, 2 * C], BF16, isOutput=False)
    wct = nc.declare_dram_parameter("wct", [8, P, 4 * 2 * 512], BF16, isOutput=False)
    identb = nc.declare_dram_parameter("identb", [P, P], BF16, isOutput=False)
    idpair8 = nc.declare_dram_parameter("idpair8", [P, 2 * P], F8, isOutput=False)
    maskd = nc.declare_dram_parameter("maskd", [4, P, 2 * 512], F8, isOutput=False)
    out = nc.declare_dram_parameter("out", [R, C], F32, isOutput=True)

    with tile.TileContext(nc) as tc, ExitStack() as ctx:
        const = ctx.enter_context(tc.tile_pool(name="const", bufs=1))
        big = ctx.enter_context(tc.tile_pool(name="big", bufs=1))
        wcol = ctx.enter_context(tc.tile_pool(name="wcol", bufs=3))
        wrow = ctx.enter_context(tc.tile_pool(name="wrow", bufs=2))
        wctp = ctx.enter_context(tc.tile_pool(name="wctp", bufs=1))
        kv = ctx.enter_context(tc.tile_pool(name="kv", bufs=2))
        sm = ctx.enter_context(tc.tile_pool(name="sm", bufs=4))
        ev = ctx.enter_context(tc.tile_pool(name="ev", bufs=3))
        esp = ctx.enter_context(tc.tile_pool(name="esp", bufs=4))
        mtp = ctx.enter_context(tc.tile_pool(name="mtp", bufs=1))
        ps = ctx.enter_context(tc.tile_pool(name="ps", bufs=6, space="PSUM"))
        dram = ctx.enter_context(tc.tile_pool(name="dram", bufs=1, space="DRAM"))

        # ---------- constants ----------
        idtb = const.tile([P, P], BF16, name="idtb")
        nc.sync.dma_start(idtb[:], identb[:])
        idp8 = const.tile([P, 2, P], F8, name="idp8")
        nc.sync.dma_start(idp8[:].rearrange("p a b -> p (a b)"), idpair8[:])
        mDz = const.tile([P, 4, 2, 512], F8, name="mDz")
        nc.sync.dma_start(
            mDz[:].rearrange("p i a c -> p i (a c)"),
            maskd[:].rearrange("i p c -> p i c"),
        )
        eps_t = const.tile([P, 1], F32, name="eps_t")
        nc.any.memset(eps_t[:], EPS)

        # ---------- collective DRAM buffers ----------
        a1q_in = dram.tile([NCORES, 2, P, R], F8, name="a1q_in")
        a1q_out = dram.tile([NCORES, 2, P, R], F8, name="a1q_out")
        a1v_in = dram.tile([NCORES, P, 2, 2, 2, Dh], F8, name="a1v_in")
        a1v_out = dram.tile([NCORES, P, 2, 2, 2, Dh], F8, name="a1v_out")
        a2_in = dram.tile([NCORES, P, R], F85, name="a2_in")
        a2_out = dram.tile([NCORES, P, R], F85, name="a2_out")

        # ---------- phase 1: load x, LN1 -> h (bf16), transpose -> hT8 ----------
        with nc.named_scope("ln1"):
            xres = big.tile([P, RT, C], F32, tag="x", name="xres")
            nc.sync.dma_start(xres[:].rearrange("p m c -> p (m c)"), xin[:])
            h = big.tile([P, RT, C], BF16, tag="h", name="h")
            for m in range(RT):
                _ln_tile(nc, sm, h[:, m, :], xres[:, m, :], eps_t)
            hT8 = big.tile([P, CK, R], F8, tag="ht8", name="hT8")
            for k in range(CK):
                pt = ps.tile([P, 512], BF16, tag="ps")
                for m in range(RT):
                    nc.tensor.matmul(
                        pt[:, m * P:(m + 1) * P],
                        h[:, m, k * P:(k + 1) * P],
                        idtb[:],
                        is_transpose=True,
                        start=(m == 0), stop=(m == RT - 1),
                    )
                nc.scalar.copy(hT8[:, k, :], pt[:])

        # ---------- phase 2: q/k matmuls (fp8 DR) -> a2a#1 ----------
        with nc.named_scope("qk"):
            for m in range(16):
                wb = wcol.tile([P, 4, 2, P], F8, tag="wc", name="wb")
                nc.sync.dma_start(
                    wb[:].rearrange("p i a q -> p (i a q)"), wqk8[m]
                )
                pm = ps.tile([P, 512], F32, tag="ps")
                for kp in range(4):
                    nc.tensor.matmul(
                        pm[:], wb[:, kp, :, :], hT8[:, 2 * kp:2 * kp + 2, :],
                        start=(kp == 0), stop=(kp == 3),
                        perf_mode=DR,
                    )
                et = ev.tile([P, 512], F8, tag="ev", name="et")
                nc.vector.tensor_copy(et[:], pm[:])
                nc.sync.dma_start(a1q_in[m % 8, m // 8], et[:])

            nc.gpsimd.collective_compute(
                "AllToAll",
                mybir.AluOpType.bypass,
                ins=[a1q_in[:].opt()],
                outs=[a1q_out[:].opt()],
                replica_groups=[list(range(NCORES))],
            )

        # ---------- phase 3: V (natural layout, plain fp8) -> a2a#2 ----------
        with nc.named_scope("v"):
            for half in range(2):
                wv = wrow.tile([P, CK, 512], F8, tag="wr", name="wv")
                nc.sync.dma_start(
                    wv[:].rearrange("p k c -> p (k c)"), wv8[half]
                )
                for m in range(RT):
                    pv = ps.tile([P, 512], F32, tag="ps")
                    for k in range(CK):
                        nc.tensor.matmul(
                            pv[:], hT8[:, k, m * P:(m + 1) * P], wv[:, k, :],
                            start=(k == 0), stop=(k == CK - 1),
                        )
                    vev = ev.tile([P, 512], F8, tag="ev", name="vev")
                    nc.vector.tensor_copy(vev[:], pv[:])
                    # scatter to dest shards: vcols 512*half+128*dq+64*hh+d
                    for dq in range(4):
                        d = 4 * half + dq
                        for hh in range(2):
                            nc.sync.dma_start(
                                a1v_in[d, :, hh, m // 2, m % 2, :],
                                vev[:, 128 * dq + 64 * hh:128 * dq + 64 * hh + 64],
                            )

        nc.gpsimd.collective_compute(
            "AllToAll",
            mybir.AluOpType.bypass,
            ins=[a1v_in[:].opt()],
            outs=[a1v_out[:].opt()],
            replica_groups=[list(range(NCORES))],
        )

        # ---------- phase 4: attention (4 (batch, head) pairs per core) ----
        # cproj weights are prefetched into held SBUF tiles between pairs so
        # the DMA runs under attention compute without blocking the kt/qt/vv
        # loads queued ahead of it.
        wcts = [
            wctp.tile([P, 4, 2, 512], BF16, tag=f"wct{i}", name=f"wct{i}")
            for i in range(8)
        ]
        wfb = [
            wcol.tile([P, 2 * C], BF16, tag="wfpre", bufs=3, name=f"wf{i}")
            for i in range(3)
        ]
        with nc.named_scope("attn"):
            for p_i in range(4):
                b = p_i // 2
                hl = p_i % 2
                sb = 4 * b
                # kt: [32, strip 16, half 2, 128] from K regions of 4 shards
                kt = kv.tile([32, 16, 2, P], F8, tag="kt", bufs=1, name="kt")
                # qt: [32, jq 4, half 2, 512] from Q regions
                qt = kv.tile([32, 4, 2, R], F8, tag="qt", bufs=1, name="qt")
                for s in range(4):
                    ksrc = a1q_out[sb + s, 1].rearrange(
                        "(hh half p) r -> hh half p r", hh=2, half=2
                    )[hl]
                    nc.sync.dma_start(
                        kt[:, 4 * s:4 * s + 4, :, :],
                        ksrc.rearrange("half p (cs r2) -> p cs half r2", cs=4),
                    )
                    qsrc = a1q_out[sb + s, 0].rearrange(
                        "(hh half p) r -> hh half p r", hh=2, half=2
                    )[hl]
                    nc.sync.dma_start(
                        qt[:, s, :, :], qsrc.rearrange("half p r -> p half r"),
                    )
                # vv: [128, shard 4, cspair 2, {cs2 2, 64 dims | ones}]
                # loaded on the DVE queue: it waits on a2a#2, and must not
                # stall the SP queue (weight prefetches, next pair's kt/qt)
                vv = kv.tile([P, 4, 2, 2, P], F8, tag="vv", bufs=1, name="vv")
                nc.gpsimd.memset(vv[:].rearrange("p s a b x -> p (s a b x)"), 0.0)
                nc.gpsimd.memset(vv[:, :, :, :, Dh:Dh + 1], 1.0)
                for s in range(4):
                    nc.gpsimd.dma_start(
                        vv[:, s, :, :, 0:Dh], a1v_out[sb + s, :, hl],
                    )

                # prefetch MLP weights under this pair's compute
                nc.sync.dma_start(wcts[2 * p_i][:].rearrange("p i h c -> p (i h c)"),
                                  wct[2 * p_i])
                nc.sync.dma_start(wcts[2 * p_i + 1][:].rearrange("p i h c -> p (i h c)"),
                                  wct[2 * p_i + 1])
                if p_i < 3:
                    nc.sync.dma_start(wfb[p_i][:], wft[p_i])

                for jq in range(4):
                    py = ps.tile([P, 512], F32, tag="av", bufs=2)
                    ngp = 2 * jq + 2
                    for gp in range(ngp):
                        es2 = esp.tile([P, 2, 512], F8, tag="es", name="es2")
                        for g2 in range(2):
                            g = 2 * gp + g2
                            pS = ps.tile([P, 512], F32, tag="ps")
                            masked = g >= 4 * jq
                            if masked:
                                nc.tensor.matmul(
                                    pS[:], idp8[:], mDz[:, g - 4 * jq, :, :],
                                    start=True, stop=False, perf_mode=DR,
                                )
                            nc.tensor.matmul(
                                pS[:],
                                kt[:, g, :, :],
                                qt[:, jq, :, :],
                                start=not masked, stop=True, perf_mode=DR,
                            )
                            nc.scalar.activation(
                                out=es2[:, g2, :], in_=pS[:],
                                func=mybir.ActivationFunctionType.Exp,
                                scale=SCALE,
                            )
                        nc.tensor.matmul(
                            py[:], vv[:, gp // 2, gp % 2, :, :], es2[:],
                            start=(gp == 0), stop=(gp == ngp - 1),
                            perf_mode=DR,
                        )
                    # normalize: y[d, q] / denom[q] (denom = psum row Dh)
                    ysf = sm.tile([Dh + 1, 512], F32, tag="ys", bufs=2)
                    nc.vector.tensor_copy(ysf[:], py[0:Dh + 1, :])
                    rr0 = sm.tile([1, 512], F32, tag="rr", bufs=2)
                    nc.gpsimd.dma_start(rr0[:], ysf[Dh:Dh + 1, :])
                    nc.vector.reciprocal(out=rr0[:], in_=rr0[:])
                    bb = sm.tile([Dh, 512], F32, tag="bb", bufs=2)
                    nc.gpsimd.partition_broadcast(bb[:], rr0[:], channels=Dh)
                    yst = sm.tile([Dh, 512], F85, tag="yst", bufs=2)
                    nc.vector.tensor_mul(yst[:], ysf[0:Dh, :], bb[:])
                    d = 4 * b + jq
                    nc.gpsimd.dma_start(
                        a2_in[d, Dh * hl:Dh * (hl + 1), :], yst[:]
                    )

        # ---------- phase 5: AllToAll #3 (yT back to row owners) ----------
        nc.gpsimd.collective_compute(
            "AllToAll",
            mybir.AluOpType.bypass,
            ins=[a2_in[:].opt()],
            outs=[a2_out[:].opt()],
            replica_groups=[list(range(NCORES))],
        )

        # ---------- phase 6: proj (fp8e5 DR) + residual into xres ----------
        with nc.named_scope("proj"):
            # yTm8: [p, m 4, kpair 4, k2 2, r2 128] fp8e5
            yTm8 = big.tile([P, RT, 4, 2, P], F85, tag="yt", name="yTm8")
            for s in range(NCORES):
                nc.sync.dma_start(
                    yTm8[:, :, s // 2, s % 2, :],
                    a2_out[s].rearrange("p (m r2) -> p m r2", m=RT),
                )
            wpb = wrow.tile([P, 4, 2, 2, 512], F85, tag="wp", bufs=1, name="wpb")
            nc.sync.dma_start(
                wpb[:].rearrange("p i h a c -> p (i h) (a c)"),
                wp8[:].rearrange("i p c -> p i c"),
            )
            for m in range(RT):
                for half in range(2):
                    pp = ps.tile([P, 512], F32, tag="ps")
                    for kp in range(4):
                        nc.tensor.matmul(
                            pp[:], yTm8[:, m, kp, :, :], wpb[:, kp, half, :, :],
                            start=(kp == 0), stop=(kp == 3), perf_mode=DR,
                        )
                    nc.vector.tensor_add(
                        xres[:, m, 512 * half:512 * half + 512], pp[:],
                        xres[:, m, 512 * half:512 * half + 512],
                    )

        # ---------- phase 7: LN2 + transpose (bf16) ----------
        with nc.named_scope("ln2"):
            h2 = big.tile([P, RT, C], BF16, tag="h", name="h2")
            for m in range(RT):
                _ln_tile(nc, sm, h2[:, m, :], xres[:, m, :], eps_t)
            h2T = big.tile([P, CK, R], BF16, tag="h2t", name="h2T")
            for k in range(CK):
                pt = ps.tile([P, 512], BF16, tag="ps")
                for m in range(RT):
                    nc.tensor.matmul(
                        pt[:, m * P:(m + 1) * P],
                        h2[:, m, k * P:(k + 1) * P],
                        idtb[:],
                        is_transpose=True,
                        start=(m == 0), stop=(m == RT - 1),
                    )
                nc.vector.tensor_copy(h2T[:, k, :], pt[:])

        # ---------- phase 8: fc (bf16) + relu -> mt (SBUF-resident) -------
        with nc.named_scope("mlp"):
            mts = []
            for m in range(32):
                if m < 6:
                    wb = wfb[m // 2]
                elif m % 2 == 0:
                    wb = wcol.tile([P, 2 * C], BF16, tag="wfpre", bufs=3, name="wbf")
                    nc.sync.dma_start(wb[:], wft[m // 2])
                wbv = wb[:].rearrange("p (i k q) -> p i k q", i=2, k=CK)
                pm = ps.tile([P, 512], F32, tag="ps")
                for k in range(CK):
                    nc.tensor.matmul(
                        pm[:], wbv[:, m % 2, k, :], h2T[:, k, :],
                        start=(k == 0), stop=(k == CK - 1),
                    )
                mtb = mtp.tile([P, 512], BF16, tag=f"mt{m}", name=f"mt{m}")
                nc.scalar.activation(
                    out=mtb[:], in_=pm[:],
                    func=mybir.ActivationFunctionType.Relu,
                )
                mts.append(mtb)

            # ---------- phase 9: cproj (bf16) + residual -> out ----------
            # weights already resident in wcts; one psum group per (m, half)
            out_r = out[:].rearrange("(m p) c -> p m c", p=P)
            for m in range(RT):
                for half in range(2):
                    pc = ps.tile([P, 512], F32, tag="ps")
                    for k in range(32):
                        nc.tensor.matmul(
                            pc[:],
                            mts[k][:, m * P:(m + 1) * P],
                            wcts[k // 4][:, k % 4, half, :],
                            start=(k == 0), stop=(k == 31),
                        )
                    ot = ev.tile([P, 512], F32, tag="ot", bufs=2, name="ot")
                    nc.vector.tensor_add(
                        ot[:], pc[:],
                        xres[:, m, 512 * half:512 * half + 512]
                    )
                    nc.sync.dma_start(
                        out_r[:, m, 512 * half:512 * half + 512], ot[:]
                    )

    nc.finalize()
    return nc


def _get_nc():
    if "nc" not in _CACHE:
        _CACHE["nc"] = build()
    return _CACHE["nc"]


def _make_in_maps(x, ln1_w, w_attn, w_proj, ln2_w, w_fc, w_cproj):
    import ml_dtypes
    E4 = ml_dtypes.float8_e4m3
    E5 = ml_dtypes.float8_e5m2
    BF = ml_dtypes.bfloat16

    x = np.asarray(x, dtype=np.float32)
    ln1_w = np.asarray(ln1_w, dtype=np.float32)
    ln2_w = np.asarray(ln2_w, dtype=np.float32)
    w_attn = np.asarray(w_attn, dtype=np.float32) * ln1_w[:, None]
    w_proj = np.asarray(w_proj, dtype=np.float32)
    w_fc = np.asarray(w_fc, dtype=np.float32) * ln2_w[:, None]
    w_cproj = np.asarray(w_cproj, dtype=np.float32)

    identb = np.eye(P, dtype=np.float32).astype(BF)
    idp = np.concatenate([np.eye(P, dtype=np.float32)] * 2, axis=1).astype(E4)
    # mask pairs: [i][p][slot 2][512]; slot0 = mask (0 / -448), slot1 = 0
    ii = np.arange(P)[:, None]
    jj = np.arange(512)[None, :]
    maskd = np.zeros((4, P, 2, 512), dtype=np.float32)
    for i in range(4):
        maskd[i, :, 0, :] = np.where(ii <= jj - P * i, 0.0, MASKV)
    maskd = maskd.reshape(4, P, 1024).astype(E4)

    # wqk8[m, kp, p, (k2 q)] = w_attn[128*(2kp+k2) + p, 128m + q]  (fp8 e4m3)
    wqk = w_attn[:, 0:2 * C]
    wqk8 = np.ascontiguousarray(
        wqk.reshape(4, 2, P, 16, P).transpose(3, 0, 2, 1, 4).reshape(16, 4, P, 2 * P)
    ).astype(E4)
    # wv8[half, p, k, c] = w_attn[128k + p, 2048 + 512*half + c]
    wv_ = w_attn[:, 2 * C:3 * C]
    wv8 = np.ascontiguousarray(
        wv_.reshape(CK, P, 2, 512).transpose(2, 1, 0, 3)
    ).astype(E4)
    # wp8[(kp*2+half), p, (k2 c)] = w_proj[128*(2kp+k2)+p, 512half+c] (e5m2)
    wp8 = np.ascontiguousarray(
        w_proj.reshape(4, 2, P, 2, 512).transpose(0, 3, 2, 1, 4).reshape(8, P, 2 * 512)
    ).astype(E5)
    # wft[m, p, (i k q)]: i in {0,1} m-subtile, k = C chunk, q = 128 fc cols
    wft = np.ascontiguousarray(
        w_fc.reshape(CK, P, 32, P).transpose(2, 1, 0, 3).reshape(16, 2, P, CK * P)
        .transpose(0, 2, 1, 3).reshape(16, P, 2 * C)
    ).astype(BF)
    # wct[kg, p, (k4 half c)] = w_cproj[128*(4kg+k4)+p, 512*half+c]
    wct = np.ascontiguousarray(
        w_cproj.reshape(8, 4, P, 2, 512).transpose(0, 2, 1, 3, 4).reshape(8, P, 4 * 2 * 512)
    ).astype(BF)

    in_maps = []
    for c in range(NCORES):
        b = c // 4
        r0 = 512 * (c % 4)
        xr = x[b, r0:r0 + R]  # [512, 1024]
        xt = np.ascontiguousarray(
            xr.reshape(RT, P, C).transpose(1, 0, 2).reshape(P, RT * C)
        )
        in_maps.append({
            "xin": xt,
            "wqk8": wqk8, "wv8": wv8, "wp8": wp8, "wft": wft, "wct": wct,
            "identb": identb, "idpair8": idp, "maskd": maskd,
        })
    return in_maps


def run(x, ln1_w, w_attn, w_proj, ln2_w, w_fc, w_cproj, trace=False):
    nc = _get_nc()
    in_maps = _make_in_maps(x, ln1_w, w_attn, w_proj, ln2_w, w_fc, w_cproj)
    res = run_bass_kernel_spmd(nc, in_maps, list(range(NCORES)), trace=trace)
    out = np.empty((B, T, C), dtype=np.float32)
    for c in range(NCORES):
        b = c // 4
        r0 = 512 * (c % 4)
        out[b, r0:r0 + R] = res.results[c]["out"]
    return out, res


def kernel(x, ln1_w, w_attn, w_proj, ln2_w, w_fc, w_cproj):
    out, _ = run(x, ln1_w, w_attn, w_proj, ln2_w, w_fc, w_cproj)
    return out
